# revision 15
# baseline (speedup 1.0000x reference)
"""Trainium2 Bass kernel: row-softmax + embedding gather (batched lookup).

reference:
    probs = softmax(poi_freq_matrix, axis=1)        # [100000, 168] f32
    out   = probs[inputs_wekn]                      # [1024, 200, 168] f32

Strategy (8 NeuronCores, data-parallel over batch; each core owns 128
batch rows = 128 SBUF partitions x 200 seq positions). All positions go
through quad dma_gather: the table is packed into [25000, 4x192] bf16
"quad" rows so int16 indices (wekn//4) cover all 100000 rows; sub-row
wekn%4 is selected in place by 3 predicated copies (DVE, bf16).

The binding resource is GpSimd (Pool): the gather ucode emits
descriptors at ~7.5ns/index, so 25600 lookups/core ≈ 192us of Pool no
matter how they're batched. Everything else is engineered under that:
 - prepare_only + trigger_dma (emission only on Pool; drains ride
   SWDGE queues 1..3 concurrently). Consumers gate on the prep's own
   completion sem (Tile's DMASW tick for preps is a pre-bump with no
   data sync).
 - m=25 positions per gather (3200 idxs) amortizes the ~1us fixed +
   ~1.4us Tile pre-bump + trigger per chunk. single_packet=False
   (single_packet caps at 64 descs/engine = m<=5... minus sem desc).
 - bf16 quads (1536B/lookup) keep the 4x-inflated reads at ~39MB/core,
   under the ~54us/chunk Pool emission time across 16 SDMA engines.
 - softmax tail: ACT exp (bf16 -> f32), DVE reduce/recip/scale, HWDGE
   store. Only the logits are bf16-quantized: ~1e-2 rel err vs 2e-2.
"""

import sys

import numpy as np

sys.path.insert(0, "/opt/trn_rl_repo")

N_POI = 100000
N_BINS = 168
DP = 192  # padded row length in bf16 elems (384B)
NQ = N_POI // 4  # quad rows
BATCH = 1024
SEQ = 200
N_CORES = 8
BPC = BATCH // N_CORES  # batch rows per core = 128 partitions

M = 25  # seq positions per quad dma_gather op
NI = 25  # tail positions via the indirect train
KI = 5  # tail positions per indirect compute group

_NC_CACHE = {}


def build(seq=SEQ, m=M, ni=NI, ki=KI, nqueues=4, scratch=32768, tbufs=3, pbufs=3):
    """Build the per-core Bass program (SPMD: same NEFF on all cores)."""
    import concourse.bacc as bacc
    import concourse.tile as tile
    from concourse import bass, mybir

    sg = seq - ni  # positions via quad gather
    assert sg % m == 0 and ni % ki == 0
    nch = sg // m
    nidx = BPC * m
    nc = bacc.Bacc(
        "TRN2",
        target_bir_lowering=False,
        debug=False,
        enable_asserts=False,
        num_devices=N_CORES,
        num_swdge_queues=nqueues,
        dynamic_dma_scratch_size=scratch,
        enable_partition_id=False,
    )
    qtab = nc.dram_tensor(
        "qtab", [NQ, 4 * DP], mybir.dt.bfloat16, kind="ExternalInput"
    ).ap()
    table = nc.dram_tensor(
        "table", [N_POI, N_BINS], mybir.dt.bfloat16, kind="ExternalInput"
    ).ap()
    widx = nc.dram_tensor(
        "widx", [128, sg * 8], mybir.dt.int16, kind="ExternalInput"
    ).ap()
    idx = nc.dram_tensor("idx", [BPC, ni], mybir.dt.int32, kind="ExternalInput").ap()
    msk = nc.dram_tensor(
        "msk", [BPC, 3 * sg], mybir.dt.uint8, kind="ExternalInput"
    ).ap()
    out = nc.dram_tensor(
        "out", [BPC, seq, N_BINS], mybir.dt.float32, kind="ExternalOutput"
    ).ap()

    with tile.TileContext(nc) as tc:
        with tc.tile_pool(name="const", bufs=1) as cpool, tc.tile_pool(
            name="quad", bufs=tbufs
        ) as tpool, tc.tile_pool(name="prob", bufs=pbufs) as ppool, tc.tile_pool(
            name="small", bufs=8
        ) as smpool:
            wt = cpool.tile([128, sg * 8], mybir.dt.int16)
            # chunk 0's idx slice loads first so its prep starts ASAP
            nc.sync.dma_start(out=wt[:, : m * 8], in_=widx[:, : m * 8])
            nc.sync.dma_start(out=wt[:, m * 8 :], in_=widx[:, m * 8 :])
            mt = cpool.tile([BPC, 3 * sg], mybir.dt.uint8)
            nc.sync.dma_start(out=mt[:], in_=msk[:])
            m3 = mt[:].rearrange("p (q s) -> p q s", q=3)
            idx_t = cpool.tile([BPC, ni], mybir.dt.int32)
            nc.sync.dma_start(out=idx_t[:], in_=idx[:])

            for c in range(nch):
                T = tpool.tile([BPC, m * 4 * DP], mybir.dt.bfloat16, tag="T")
                T4 = T[:].rearrange("p (m q d) -> p m q d", m=m, q=4)
                q = 1 + c % (nqueues - 1)
                gsem = nc.alloc_semaphore(f"gsem{c}")
                nc.gpsimd.dma_gather(
                    out_ap=T[:].rearrange("p (m d) -> p m d", m=m),
                    in_ap=qtab[:],
                    idxs_ap=wt[:, c * m * 8 : (c + 1) * m * 8],
                    num_idxs=nidx,
                    num_idxs_reg=nidx,
                    elem_size=4 * DP,
                    elem_step=4 * DP,
                    single_packet=False,
                    prepare_only=True,
                    sem=gsem,
                    queue_num=q,
                )
                nc.gpsimd.trigger_dma(count=None, queue_num=q)
                # prep data flow is user-synced: gate the select on the
                # prep's completion sem (Tile's DMASW tick for preps is a
                # pre-bump with no data sync)
                nc.vector.wait_ge(gsem, 16)
                # select as uint32 pairs: halves DVE elems (and the SBUF
                # port pressure that slows the Q7's descriptor emission)
                Tu = T[:].bitcast(mybir.dt.uint32)
                U4 = Tu.rearrange("p (m q d) -> p m q d", m=m, q=4)
                selu = U4[:, :, 0, : N_BINS // 2]
                for qq in (1, 2, 3):
                    nc.vector.copy_predicated(
                        out=selu,
                        mask=m3[:, qq - 1, c * m : (c + 1) * m].to_broadcast(
                            [BPC, m, N_BINS // 2]
                        ),
                        data=U4[:, :, qq, : N_BINS // 2],
                    )
                sel = T4[:, :, 0, :N_BINS]
                P = ppool.tile([BPC, m * N_BINS], mybir.dt.float32, tag="P")
                P3 = P[:].rearrange("p (m d) -> p m d", m=m)
                nc.scalar.activation(
                    out=P3, in_=sel, func=mybir.ActivationFunctionType.Exp
                )
                sums = smpool.tile([BPC, m], mybir.dt.float32, tag="sums")
                nc.vector.tensor_reduce(
                    out=sums[:],
                    in_=P3,
                    axis=mybir.AxisListType.X,
                    op=mybir.AluOpType.add,
                )
                rec = smpool.tile([BPC, m], mybir.dt.float32, tag="rec")
                nc.vector.reciprocal(out=rec[:], in_=sums[:])
                # scale on ACT (per position; scale is a [128,1] AP) to
                # keep the 2-port tensor_tensor off DVE
                for j in range(m):
                    nc.scalar.activation(
                        out=P3[:, j],
                        in_=P3[:, j],
                        func=mybir.ActivationFunctionType.Copy,
                        scale=rec[:, j : j + 1],
                    )
                nc.sync.dma_start(out=out[:, c * m : (c + 1) * m, :], in_=P[:])

            # indirect train for the tail positions: runs on Pool while
            # the last gather chunk drains and its softmax tail computes
            for ci in range(ni // ki):
                g = ppool.tile([BPC, ki * N_BINS], mybir.dt.bfloat16, tag="g")
                g3 = g[:].rearrange("p (k d) -> p k d", k=ki)
                for j in range(ki):
                    nc.gpsimd.indirect_dma_start(
                        out=g[:, j * N_BINS : (j + 1) * N_BINS],
                        out_offset=None,
                        in_=table[:],
                        in_offset=bass.IndirectOffsetOnAxis(
                            ap=idx_t[:, ci * ki + j : ci * ki + j + 1], axis=0
                        ),
                    )
                G = ppool.tile([BPC, ki * N_BINS], mybir.dt.float32, tag="G")
                G3 = G[:].rearrange("p (k d) -> p k d", k=ki)
                nc.scalar.activation(
                    out=G3, in_=g3, func=mybir.ActivationFunctionType.Exp
                )
                sums2 = smpool.tile([BPC, ki], mybir.dt.float32, tag="s2")
                nc.vector.tensor_reduce(
                    out=sums2[:],
                    in_=G3,
                    axis=mybir.AxisListType.X,
                    op=mybir.AluOpType.add,
                )
                rec2 = smpool.tile([BPC, ki], mybir.dt.float32, tag="r2")
                nc.vector.reciprocal(out=rec2[:], in_=sums2[:])
                for j in range(ki):
                    nc.scalar.activation(
                        out=G3[:, j],
                        in_=G3[:, j],
                        func=mybir.ActivationFunctionType.Copy,
                        scale=rec2[:, j : j + 1],
                    )
                nc.sync.dma_start(
                    out=out[:, sg + ci * ki : sg + (ci + 1) * ki, :], in_=G[:]
                )
    nc.compile()
    return nc


def _prep_inputs(wekn, table, seq=SEQ, m=M, ni=NI):
    """Host-side layout/index prep: bf16 cast, padded quad table, wrapped
    int16 quad ids, sub-row masks, per-core shards."""
    import ml_dtypes

    tb = np.ascontiguousarray(table.astype(ml_dtypes.bfloat16))
    qt = np.zeros((NQ, 4, DP), dtype=ml_dtypes.bfloat16)
    qt[:, :, :N_BINS] = tb.reshape(NQ, 4, N_BINS)
    qt = np.ascontiguousarray(qt.reshape(NQ, 4 * DP))
    sg = seq - ni
    nch = sg // m
    in_maps = []
    for core in range(N_CORES):
        wc = wekn[core * BPC : (core + 1) * BPC]
        wq = wc[:, :sg]
        quad = (wq // 4).astype(np.int16)
        sub = wq % 4
        wi = np.empty((16, sg * 8), dtype=np.int16)
        for c in range(nch):
            walk = quad[:, c * m : (c + 1) * m].T.reshape(-1)
            wi[:, c * m * 8 : (c + 1) * m * 8] = walk.reshape(m * 8, 16).T
        mk = np.empty((BPC, 3, sg), dtype=np.uint8)
        for qq in (1, 2, 3):
            mk[:, qq - 1] = (sub == qq).astype(np.uint8)
        in_maps.append(
            {
                "qtab": qt,
                "table": tb,
                "widx": np.tile(wi, (8, 1)),
                "idx": np.ascontiguousarray(wc[:, sg:].astype(np.int32)),
                "msk": np.ascontiguousarray(mk.reshape(BPC, 3 * sg)),
            }
        )
    return in_maps


def _get_nc():
    if "nc" not in _NC_CACHE:
        _NC_CACHE["nc"] = build()
    return _NC_CACHE["nc"]


def kernel(**inputs) -> np.ndarray:
    wekn = np.asarray(inputs["inputs_wekn"]).astype(np.int64)
    table = np.ascontiguousarray(
        np.asarray(inputs["poi_freq_matrix"], dtype=np.float32)
    )
    assert wekn.shape == (BATCH, SEQ) and table.shape == (N_POI, N_BINS)

    from concourse.bass_utils import run_bass_kernel_spmd

    nc = _get_nc()
    in_maps = _prep_inputs(wekn, table)
    res = run_bass_kernel_spmd(nc, in_maps, core_ids=list(range(N_CORES)))
    return np.concatenate([res.results[c]["out"] for c in range(N_CORES)], axis=0)


if __name__ == "__main__":
    rng = np.random.default_rng(0)
    inputs = {
        "venueid2coor": rng.random((N_POI, 2), dtype=np.float32),
        "inputs_wekn": rng.integers(0, N_POI, size=(BATCH, SEQ), dtype=np.int64),
        "poi_freq_matrix": rng.standard_normal((N_POI, N_BINS), dtype=np.float32),
    }
    out = kernel(**inputs)
    print(out.shape, out.dtype)


# revision 16
# speedup vs baseline: 1.2234x; 1.2234x over previous
"""Trainium2 Bass kernel: row-softmax + embedding gather (batched lookup).

reference:
    probs = softmax(poi_freq_matrix, axis=1)        # [100000, 168] f32
    out   = probs[inputs_wekn]                      # [1024, 200, 168] f32

Strategy (8 NeuronCores, data-parallel over batch; each core owns 128
batch rows = 128 SBUF partitions x 200 seq positions). All positions go
through quad dma_gather: the table is packed into [25000, 4x192] bf16
"quad" rows so int16 indices (wekn//4) cover all 100000 rows; sub-row
wekn%4 is selected in place by 3 predicated copies.

The binding resource is GpSimd (Pool): the gather ucode emits
descriptors at ~8ns/index, so 25600 lookups/core ~ 205us of Pool no
matter how they're batched. Everything else is engineered under that:
 - big chunks (m=25 positions, 3200 idxs per dma_gather) amortize the
   ~1us fixed cost; the last two chunks taper (15, 10) so the final
   drain+softmax tail is short. single_packet=False (single_packet
   caps at 64 descs/engine incl the sem desc).
 - all gathers ride ONE SWDGE queue so Tile's DMASW lane sems are
   touched from a single queue (the ucode locks each sem to a queue);
   the queue's 16 engine-rings drain a chunk (~15us) faster than the
   next emission (~26us), so the single queue never stalls the Q7.
 - bf16 quads (1536B/lookup): gather reads ~39MB/core, hidden under
   emission across 16 SDMA engines. Only the logits are quantized:
   ~1e-2 rel err vs the 2e-2 gate.
 - the select runs as uint32-pair predicated copies (84 elems/row,
   half the DVE time and SBUF-port pressure -- DVE shares the POOL
   port with the Q7's descriptor-ring writes, which slows emission).
 - softmax tail: ACT exp (bf16 -> f32), DVE reduce + reciprocal, and
   the scale multiply on ACT via per-position activation(Copy,
   scale=[128,1]) to keep 2-port tensor_tensor off DVE. HWDGE store.
"""

import sys

import numpy as np

sys.path.insert(0, "/opt/trn_rl_repo")

N_POI = 100000
N_BINS = 168
DP = 192  # padded row length in bf16 elems (384B)
NQ = N_POI // 4  # quad rows
BATCH = 1024
SEQ = 200
N_CORES = 8
BPC = BATCH // N_CORES  # batch rows per core = 128 partitions

CHUNKS = (25, 25, 25, 25, 25, 25, 25, 15, 10)  # seq positions per gather

_NC_CACHE = {}


def build(chunks=CHUNKS, nqueues=2, scratch=32768, tbufs=3, pbufs=3):
    """Build the per-core Bass program (SPMD: same NEFF on all cores)."""
    import concourse.bacc as bacc
    import concourse.tile as tile
    from concourse import bass, mybir

    seq = sum(chunks)
    mx = max(chunks)
    nc = bacc.Bacc(
        "TRN2",
        target_bir_lowering=False,
        debug=False,
        enable_asserts=False,
        num_devices=N_CORES,
        num_swdge_queues=nqueues,
        dynamic_dma_scratch_size=scratch,
        enable_partition_id=False,
    )
    qtab = nc.dram_tensor(
        "qtab", [NQ, 4 * DP], mybir.dt.bfloat16, kind="ExternalInput"
    ).ap()
    widx = nc.dram_tensor(
        "widx", [128, seq * 8], mybir.dt.int16, kind="ExternalInput"
    ).ap()
    msk = nc.dram_tensor(
        "msk", [BPC, 3 * seq], mybir.dt.uint8, kind="ExternalInput"
    ).ap()
    out = nc.dram_tensor(
        "out", [BPC, seq, N_BINS], mybir.dt.float32, kind="ExternalOutput"
    ).ap()

    with tile.TileContext(nc) as tc:
        with tc.tile_pool(name="const", bufs=1) as cpool, tc.tile_pool(
            name="quad", bufs=tbufs
        ) as tpool, tc.tile_pool(name="prob", bufs=pbufs) as ppool, tc.tile_pool(
            name="small", bufs=8
        ) as smpool:
            m0 = chunks[0]
            wt = cpool.tile([128, seq * 8], mybir.dt.int16)
            # chunk 0's idx slice loads first so its gather starts ASAP
            nc.sync.dma_start(out=wt[:, : m0 * 8], in_=widx[:, : m0 * 8])
            nc.sync.dma_start(out=wt[:, m0 * 8 :], in_=widx[:, m0 * 8 :])
            mt = cpool.tile([BPC, 3 * seq], mybir.dt.uint8)
            nc.sync.dma_start(out=mt[:], in_=msk[:])
            m3 = mt[:].rearrange("p (q s) -> p q s", q=3)

            off = 0
            for m in chunks:
                # full-size tiles (one tag each) sliced to this chunk's m
                Tf = tpool.tile([BPC, mx * 4 * DP], mybir.dt.bfloat16, tag="T")
                T4 = Tf[:, : m * 4 * DP].rearrange(
                    "p (m q d) -> p m q d", m=m, q=4
                )
                nc.gpsimd.dma_gather(
                    out_ap=Tf[:, : m * 4 * DP].rearrange(
                        "p (m d) -> p m d", m=m
                    ),
                    in_ap=qtab[:],
                    idxs_ap=wt[:, off * 8 : (off + m) * 8],
                    num_idxs=BPC * m,
                    num_idxs_reg=BPC * m,
                    elem_size=4 * DP,
                    elem_step=4 * DP,
                    single_packet=False,
                    queue_num=1,
                )
                # select as uint32 pairs: halves DVE elems (and the SBUF
                # port pressure that slows the Q7's descriptor emission)
                Tu = Tf[:, : m * 4 * DP].bitcast(mybir.dt.uint32)
                U4 = Tu.rearrange("p (m q d) -> p m q d", m=m, q=4)
                selu = U4[:, :, 0, : N_BINS // 2]
                for qq in (1, 2, 3):
                    nc.vector.copy_predicated(
                        out=selu,
                        mask=m3[:, qq - 1, off : off + m].to_broadcast(
                            [BPC, m, N_BINS // 2]
                        ),
                        data=U4[:, :, qq, : N_BINS // 2],
                    )
                sel = T4[:, :, 0, :N_BINS]
                Pf = ppool.tile([BPC, mx * N_BINS], mybir.dt.float32, tag="P")
                P3 = Pf[:, : m * N_BINS].rearrange("p (m d) -> p m d", m=m)
                nc.scalar.activation(
                    out=P3, in_=sel, func=mybir.ActivationFunctionType.Exp
                )
                sums = smpool.tile([BPC, mx], mybir.dt.float32, tag="sums")
                nc.vector.tensor_reduce(
                    out=sums[:, :m],
                    in_=P3,
                    axis=mybir.AxisListType.X,
                    op=mybir.AluOpType.add,
                )
                rec = smpool.tile([BPC, mx], mybir.dt.float32, tag="rec")
                nc.vector.reciprocal(out=rec[:, :m], in_=sums[:, :m])
                # scale on ACT (per position; scale is a [128,1] AP) to
                # keep the 2-port tensor_tensor off DVE
                for j in range(m):
                    nc.scalar.activation(
                        out=P3[:, j],
                        in_=P3[:, j],
                        func=mybir.ActivationFunctionType.Copy,
                        scale=rec[:, j : j + 1],
                    )
                nc.sync.dma_start(
                    out=out[:, off : off + m, :], in_=Pf[:, : m * N_BINS]
                )
                off += m
    nc.compile()
    return nc


def _prep_inputs(wekn, table, chunks=CHUNKS):
    """Host-side layout/index prep: bf16 cast, padded quad table, wrapped
    int16 quad ids, sub-row masks, per-core shards."""
    import ml_dtypes

    seq = sum(chunks)
    tb = table.astype(ml_dtypes.bfloat16)
    qt = np.zeros((NQ, 4, DP), dtype=ml_dtypes.bfloat16)
    qt[:, :, :N_BINS] = tb.reshape(NQ, 4, N_BINS)
    qt = np.ascontiguousarray(qt.reshape(NQ, 4 * DP))
    in_maps = []
    for core in range(N_CORES):
        wc = wekn[core * BPC : (core + 1) * BPC]
        quad = (wc // 4).astype(np.int16)
        sub = wc % 4
        wi = np.empty((16, seq * 8), dtype=np.int16)
        off = 0
        for m in chunks:
            walk = quad[:, off : off + m].T.reshape(-1)
            wi[:, off * 8 : (off + m) * 8] = walk.reshape(m * 8, 16).T
            off += m
        mk = np.empty((BPC, 3, seq), dtype=np.uint8)
        for qq in (1, 2, 3):
            mk[:, qq - 1] = (sub == qq).astype(np.uint8)
        in_maps.append(
            {
                "qtab": qt,
                "widx": np.tile(wi, (8, 1)),
                "msk": np.ascontiguousarray(mk.reshape(BPC, 3 * seq)),
            }
        )
    return in_maps


def _get_nc():
    if "nc" not in _NC_CACHE:
        _NC_CACHE["nc"] = build()
    return _NC_CACHE["nc"]


def kernel(**inputs) -> np.ndarray:
    wekn = np.asarray(inputs["inputs_wekn"]).astype(np.int64)
    table = np.ascontiguousarray(
        np.asarray(inputs["poi_freq_matrix"], dtype=np.float32)
    )
    assert wekn.shape == (BATCH, SEQ) and table.shape == (N_POI, N_BINS)

    from concourse.bass_utils import run_bass_kernel_spmd

    nc = _get_nc()
    in_maps = _prep_inputs(wekn, table)
    res = run_bass_kernel_spmd(nc, in_maps, core_ids=list(range(N_CORES)))
    return np.concatenate([res.results[c]["out"] for c in range(N_CORES)], axis=0)


if __name__ == "__main__":
    rng = np.random.default_rng(0)
    inputs = {
        "venueid2coor": rng.random((N_POI, 2), dtype=np.float32),
        "inputs_wekn": rng.integers(0, N_POI, size=(BATCH, SEQ), dtype=np.int64),
        "poi_freq_matrix": rng.standard_normal((N_POI, N_BINS), dtype=np.float32),
    }
    out = kernel(**inputs)
    print(out.shape, out.dtype)
